# revision 46
# baseline (speedup 1.0000x reference)
"""Trainium2 Bass kernel for nn_CrossAttention (B=8, L=2048, D=1024).

Sharding: data-parallel over batch — each of the 8 NeuronCores handles one
batch element end-to-end (no collectives).

fp8(e4m3) version: all five big matmuls (q/k/v projections, scores, PV,
gate) run in fp8 with MatmulPerfMode.DoubleRow (256-deep contraction per
instruction, ~1.5x bf16 matmul throughput). fp32 PSUM accumulation keeps
the error at ~1.6e-3 (validated against the reference in numpy):
  - weights are pre-scaled by 64 before the fp8 cast (keeps N(0,1/1024)
    entries in the e4m3 normal range); the 1/64 descale is folded into the
    PSUM-evacuation activation's scale.
  - P = exp(S/sqrt(D) - 1.5): the offset keeps exp below the TRN e4m3 max
    of 240 and cancels exactly in the softmax normalization.
All intermediates (kp^T, vp, Wg, P, x) stay SBUF-resident — no DRAM
bounce. Activations are kept feature-on-partition ("transposed") so every
matmul contracts along partitions; inputs are transposed on the PE
(identity-matmul) after an fp8 convert, and the result is transposed back
and fused with mask + residual (f32) on the way out.
"""

import numpy as np

import concourse.bass as bass
import concourse.bacc as bacc
import concourse.tile as tile
import concourse.mybir as mybir
from concourse.bass_utils import run_bass_kernel_spmd

f32 = mybir.dt.float32
bf16 = mybir.dt.bfloat16
fp8 = mybir.dt.float8e4
AF = mybir.ActivationFunctionType
ALU = mybir.AluOpType
DR = mybir.MatmulPerfMode.DoubleRow

B = 8
L = 2048
D = 1024
P = 128
NT = D // P        # 8 feature tiles
JT = L // P        # 16 key tiles
IC = 512           # query chunk (free dim of moving operands)
NCHUNK = L // IC   # 4
GROUP = IC // P    # 4 row-tiles per chunk/group
NPAIR = NT // 2    # 4 feature-tile pairs (DoubleRow contracts 256)
JPAIR = JT // 2    # 8 key-tile pairs
SCALE = 1.0 / np.sqrt(np.float32(D))
WS = 64.0          # weight pre-scale before fp8 cast
EXP_OFF = -1.5     # exp bias: keeps P under the TRN e4m3 max (240)


def build_kernel(n_iters: int = 1, hw_loop: bool = False):
    nc = bacc.Bacc("TRN2", target_bir_lowering=False, debug=False)

    q_d = nc.dram_tensor("q", [L, D], f32, kind="ExternalInput").ap()
    k_d = nc.dram_tensor("k", [L, D], f32, kind="ExternalInput").ap()
    v_d = nc.dram_tensor("v", [L, D], f32, kind="ExternalInput").ap()
    mask_d = nc.dram_tensor("mask", [L], f32, kind="ExternalInput").ap()
    Wq_d = nc.dram_tensor("Wq", [D, D], f32, kind="ExternalInput").ap()
    bq_d = nc.dram_tensor("bq", [D], f32, kind="ExternalInput").ap()
    Wk_d = nc.dram_tensor("Wk", [D, D], f32, kind="ExternalInput").ap()
    bk_d = nc.dram_tensor("bk", [D], f32, kind="ExternalInput").ap()
    Wv_d = nc.dram_tensor("Wv", [D, D], f32, kind="ExternalInput").ap()
    bv_d = nc.dram_tensor("bv", [D], f32, kind="ExternalInput").ap()
    Wg_d = nc.dram_tensor("Wg", [2 * D, D], f32, kind="ExternalInput").ap()
    bg_d = nc.dram_tensor("bg", [D], f32, kind="ExternalInput").ap()
    out_d = nc.dram_tensor("out", [L, D], f32, kind="ExternalOutput").ap()

    from contextlib import ExitStack, nullcontext
    with tile.TileContext(nc) as tc:
        with ExitStack() as stack:
            pool = lambda *a, **kw: stack.enter_context(tc.tile_pool(*a, **kw))
            cst = pool(name="cst", bufs=1)
            fw32 = pool(name="fw32", bufs=2)      # [128,1024]f32 x2 =  8KB
            natp = pool(name="nat", bufs=2)       # [128,1024]f32 x2 =  8KB
            nat8 = pool(name="nat8", bufs=4)      # [128,1024]bf16 x4 =  8KB
            ktp = pool(name="kt", bufs=2)         # [128,8,512]fp8 x2 = 8KB
            ktbp = pool(name="ktb", bufs=1)       # [128,8,512]bf16  =  8KB
            wq8p = pool(name="wq8", bufs=1)       # [128,8,1024]fp8  =  8KB
            wk8p = pool(name="wk8", bufs=1)       # 8KB
            wv8p = pool(name="wv8", bufs=1)       # 8KB
            wg8p = pool(name="wg8", bufs=1)       # [128,16,1024]fp8 = 16KB
            kpTp = pool(name="kpT", bufs=1)       # [128,8,2048]fp8  = 16KB
            vpp = pool(name="vp", bufs=1)         # [128,16,1024]fp8 = 16KB
            qpTp = pool(name="qpT", bufs=1)       # [128,8,2048]fp8  = 16KB
            ptp = pool(name="pt", bufs=2)         # [128,16,512]fp8 x2 = 16KB
            xtp = pool(name="xt", bufs=2)         # [128,8,512]fp8 x2 =  8KB
            sgp = pool(name="sg", bufs=5)         # [128,512]bf16 x5 =  5KB
            rtp = pool(name="rt", bufs=8)         # [128,512]bf16 x8 =  8KB
            rsp = pool(name="rs", bufs=1)         # [128,4,1024]bf16 =  8KB
            q0p = pool(name="q0", bufs=2)         # [128,1024]f32 x2 =  8KB
            oscp = pool(name="osb", bufs=2)       # [128,1024]f32 x2 =  8KB
            mscp = pool(name="msc", bufs=2)       # misc f32         =  ~8KB
            psmm = pool(name="ps", bufs=7, space="PSUM")   # 7 banks
            pslb = pool(name="psl", bufs=1, space="PSUM")  # colsum

            # ---- constants ----
            # ones for DoubleRow colsum: [128, 2, 16] so the pair-dim byte
            # step (16) satisfies the DoubleRow weight-AP alignment.
            ones_p8 = cst.tile([P, 2, 16], fp8, tag="ones_p8")
            nc.vector.memset(ones_p8[:], 1.0)
            mask_h = cst.tile([P, JT], f32, tag="mask_h")
            nc.sync.dma_start(mask_h[:], mask_d.rearrange("(t p) -> p t", p=P))
            # R = xn*(1+tanh) = 2*xn*sigmoid(gate), so fold the 0.5 into mask
            nc.vector.tensor_scalar_mul(mask_h[:], mask_h[:], 0.5)
            bq_t = cst.tile([P, NT], f32, tag="bq_t")
            nc.sync.dma_start(bq_t[:], bq_d.rearrange("(t p) -> p t", p=P))
            bk_t = cst.tile([P, NT], f32, tag="bk_t")
            nc.sync.dma_start(bk_t[:], bk_d.rearrange("(t p) -> p t", p=P))
            bg_h = cst.tile([P, NT], f32, tag="bg_h")
            nc.sync.dma_start(bg_h[:], bg_d.rearrange("(t p) -> p t", p=P))
            nc.vector.tensor_scalar_mul(bg_h[:], bg_h[:], 0.5)
            eoff = cst.tile([P, 1], f32, tag="eoff")
            nc.vector.memset(eoff[:], EXP_OFF)
            one_pp = cst.tile([P, 1], f32, tag="one_pp")
            nc.vector.memset(one_pp[:], 1.0)
            # bv broadcast along free dim (vp_sb is row-on-partition, so the
            # per-feature bias varies along free): bv_bc[p, d] = bv[d]
            bv_r = cst.tile([1, D], f32, tag="bv_r")
            nc.sync.dma_start(bv_r[:], bv_d.rearrange("(o d) -> o d", o=1))
            bv_bc = cst.tile([P, D], f32, tag="bv_bc")
            nc.gpsimd.partition_broadcast(bv_bc[:], bv_r[:])

            Wq8 = wq8p.tile([P, NT, D], fp8, tag="w")
            Wk8 = wk8p.tile([P, NT, D], fp8, tag="w")
            Wv8 = wv8p.tile([P, NT, D], fp8, tag="w")
            Wg8 = wg8p.tile([P, 2 * NT, D], fp8, tag="w")
            kpT_sb = kpTp.tile([P, NT, L], fp8, tag="kpT")
            vp_sb = vpp.tile([P, JT, D], fp8, tag="vp")
            qpT_sb = qpTp.tile([P, NT, L], fp8, tag="qpT")

            def load_w8(dst, w_d, ntiles):
                # DMA f32 weight rows, cast to fp8 with the x64 pre-scale
                for r in range(ntiles):
                    w32 = fw32.tile([P, D], f32, tag="fw32")
                    nc.scalar.dma_start(w32[:], w_d[r * P:(r + 1) * P, :])
                    nc.scalar.activation(dst[:, r, :], w32[:], AF.Identity,
                                         scale=WS)

            def load_group_T(src_d, g, dst_kt):
                """Rows [g*512,(g+1)*512) of src_d -> dst_kt[p, et, j] =
                fp8(src[g*512+j, et*128+p]) (feature on partition), via the
                XBAR DMA transpose (one instruction per 128-row stage) and a
                strided bf16->fp8 convert."""
                ktb = ktbp.tile([P, NT, IC], bf16, tag="ktb")
                for t in range(GROUP):
                    n32 = natp.tile([P, D], f32, tag="nat")
                    r0 = (g * GROUP + t) * P
                    nc.sync.dma_start(n32[:], src_d[r0:r0 + P, :])
                    n8 = nat8.tile([P, D], bf16, tag="nat8")
                    if t < 2:
                        nc.gpsimd.tensor_copy(n8[:], n32[:])
                    elif t == 2:
                        nc.scalar.copy(n8[:], n32[:])
                    else:
                        nc.vector.tensor_copy(n8[:], n32[:])
                    eng = nc.sync if t % 2 == 0 else nc.scalar
                    eng.dma_start_transpose(
                        ktb[:, :, t * P:(t + 1) * P], n8[:])
                    if t % 2 == 0:
                        nc.scalar.copy(dst_kt[:, :, t * P:(t + 1) * P],
                                       ktb[:, :, t * P:(t + 1) * P])
                    else:
                        nc.vector.tensor_copy(dst_kt[:, :, t * P:(t + 1) * P],
                                              ktb[:, :, t * P:(t + 1) * P])

            def body_ctx():
                if hw_loop and n_iters > 1:
                    return tc.For_i(0, n_iters, 1)
                return nullcontext()

            for _ in range(1 if hw_loop else n_iters):
              with body_ctx():
                # ========== k projection -> kpT_sb (SBUF-resident) ==========
                load_w8(Wk8, Wk_d, NT)
                for g in range(NCHUNK):
                    kt = ktp.tile([P, NT, IC], fp8, tag="kt")
                    load_group_T(k_d, g, kt)
                    for nt in range(NT):
                        ps = psmm.tile([P, IC], f32, tag="mm")
                        for m in range(NPAIR):
                            nc.tensor.matmul(
                                ps[:], Wk8[:, 2 * m:2 * m + 2, nt * P:(nt + 1) * P],
                                kt[:, 2 * m:2 * m + 2, :],
                                start=(m == 0), stop=(m == NPAIR - 1),
                                perf_mode=DR)
                        nc.scalar.activation(
                            kpT_sb[:, nt, g * IC:(g + 1) * IC], ps[:],
                            AF.Identity, bias=bk_t[:, nt:nt + 1], scale=1.0 / WS)

                # ========== v projection -> vp_sb (row on partition) ==========
                load_w8(Wv8, Wv_d, NT)
                for g in range(NCHUNK):
                    vt = ktp.tile([P, NT, IC], fp8, tag="kt")
                    load_group_T(v_d, g, vt)
                    for rt_i in range(GROUP):
                        for fh in range(2):
                            ps = psmm.tile([P, IC], f32, tag="mm")
                            for m in range(NPAIR):
                                nc.tensor.matmul(
                                    ps[:], vt[:, 2 * m:2 * m + 2, rt_i * P:(rt_i + 1) * P],
                                    Wv8[:, 2 * m:2 * m + 2, fh * IC:(fh + 1) * IC],
                                    start=(m == 0), stop=(m == NPAIR - 1),
                                    perf_mode=DR)
                            # bv folded in here: (P@(vp+bv))/l = x + bv exactly
                            nc.vector.scalar_tensor_tensor(
                                vp_sb[:, g * GROUP + rt_i, fh * IC:(fh + 1) * IC],
                                ps[:], 1.0 / WS,
                                bv_bc[:, fh * IC:(fh + 1) * IC],
                                ALU.mult, ALU.add)

                load_w8(Wg8, Wg_d, 2 * NT)

                # ========== q projection -> qpT_sb ==========
                load_w8(Wq8, Wq_d, NT)
                for g in range(NCHUNK):
                    qt = ktp.tile([P, NT, IC], fp8, tag="kt")
                    load_group_T(q_d, g, qt)
                    for nt in range(NT):
                        ps = psmm.tile([P, IC], f32, tag="mm")
                        for m in range(NPAIR):
                            nc.tensor.matmul(
                                ps[:], Wq8[:, 2 * m:2 * m + 2, nt * P:(nt + 1) * P],
                                qt[:, 2 * m:2 * m + 2, :],
                                start=(m == 0), stop=(m == NPAIR - 1),
                                perf_mode=DR)
                        nc.vector.tensor_scalar(
                            qpT_sb[:, nt, g * IC:(g + 1) * IC], ps[:],
                            1.0 / WS, bq_t[:, nt:nt + 1], ALU.mult, ALU.add)

                # ========== per-chunk fused attention ==========
                for ic in range(NCHUNK):
                    qsl = slice(ic * IC, (ic + 1) * IC)
                    # --- scores S^T + exp -> pt (fp8) ---
                    pt = ptp.tile([P, JT, IC], fp8, tag="pt")
                    for jt in range(JT):
                        ps = psmm.tile([P, IC], f32, tag="mm")
                        for m in range(NPAIR):
                            nc.tensor.matmul(
                                ps[:], kpT_sb[:, 2 * m:2 * m + 2, jt * P:(jt + 1) * P],
                                qpT_sb[:, 2 * m:2 * m + 2, qsl],
                                start=(m == 0), stop=(m == NPAIR - 1),
                                perf_mode=DR)
                        nc.scalar.activation(pt[:, jt, :], ps[:], AF.Exp,
                                             bias=eoff[:], scale=float(SCALE))

                    # --- l = colsum(P), r = 1/l, broadcast ---
                    ps_l = pslb.tile([1, IC], f32, tag="lb")
                    for jj in range(JPAIR):
                        nc.tensor.matmul(ps_l[:], ones_p8[:, :, 0:1],
                                         pt[:, 2 * jj:2 * jj + 2, :],
                                         start=(jj == 0), stop=(jj == JPAIR - 1),
                                         perf_mode=DR)
                    r_sb = mscp.tile([1, IC], f32, tag="r_sb", bufs=1)
                    nc.vector.reciprocal(r_sb[:], ps_l[:])
                    rbc = mscp.tile([P, IC], f32, tag="rbc")
                    nc.gpsimd.partition_broadcast(rbc[:], r_sb[:])

                    # --- x = (P @ (vp+bv)) * r   (bv was folded into vp) ---
                    xt8 = xtp.tile([P, NT, IC], fp8, tag="xt")
                    for dt in range(NT):
                        ps = psmm.tile([P, IC], f32, tag="mm")
                        for jj in range(JPAIR):
                            nc.tensor.matmul(
                                ps[:], vp_sb[:, 2 * jj:2 * jj + 2, dt * P:(dt + 1) * P],
                                pt[:, 2 * jj:2 * jj + 2, :],
                                start=(jj == 0), stop=(jj == JPAIR - 1),
                                perf_mode=DR)
                        nc.vector.tensor_mul(xt8[:, dt, :], ps[:], rbc[:])

                    # --- gate + tanh (sigmoid folded) ---
                    sigs = []
                    for nt in range(NT):
                        ps = psmm.tile([P, IC], f32, tag="mm")
                        for m in range(NPAIR):
                            nc.tensor.matmul(
                                ps[:], Wg8[:, 2 * m:2 * m + 2, nt * P:(nt + 1) * P],
                                qpT_sb[:, 2 * m:2 * m + 2, qsl],
                                start=(m == 0), stop=False, perf_mode=DR)
                        for m in range(NPAIR):
                            nc.tensor.matmul(
                                ps[:], Wg8[:, NT + 2 * m:NT + 2 * m + 2, nt * P:(nt + 1) * P],
                                xt8[:, 2 * m:2 * m + 2, :],
                                start=False, stop=(m == NPAIR - 1), perf_mode=DR)
                        sg = sgp.tile([P, IC], bf16, tag="sg")
                        nc.scalar.activation(sg[:], ps[:], AF.Tanh,
                                             bias=bg_h[:, nt:nt + 1], scale=0.5 / WS)
                        sigs.append(sg)

                    # --- R^T = x*(1 + tanh) = 2*x*sigmoid(gate), one stt op ---
                    rts = []
                    for mi in range(NT):
                        r_t = rtp.tile([P, IC], bf16, tag="rt")
                        nc.vector.scalar_tensor_tensor(
                            r_t[:], sigs[mi][:], one_pp[:, 0:1],
                            xt8[:, mi, :], ALU.add, ALU.mult)
                        rts.append(r_t)

                    # --- DMA-transpose back, apply mask+residual, store ---
                    # rstage[p, t, mi*128+f] = R^T[mi*128+f, t*128+p]
                    rstage = rsp.tile([P, GROUP, D], bf16, tag="rs")
                    for mi in range(NT):
                        eng = nc.sync if mi % 2 == 0 else nc.scalar
                        eng.dma_start_transpose(
                            rstage[:, :, mi * P:(mi + 1) * P], rts[mi][:])
                    for t in range(GROUP):
                        it = ic * GROUP + t
                        q0 = q0p.tile([P, D], f32, tag="q0")
                        nc.scalar.dma_start(q0[:], q_d[it * P:(it + 1) * P, :])
                        osb = oscp.tile([P, D], f32, tag="osb")
                        nc.vector.scalar_tensor_tensor(
                            osb[:], rstage[:, t, :], mask_h[:, it:it + 1],
                            q0[:], ALU.mult, ALU.add)
                        nc.gpsimd.dma_start(
                            out_d[it * P:(it + 1) * P, :], osb[:])

    nc.compile()
    return nc


_CACHE = {}


def _get_nc(n_iters=1):
    if n_iters not in _CACHE:
        _CACHE[n_iters] = build_kernel(n_iters)
    return _CACHE[n_iters]


def kernel(**inputs):
    ins = {n: np.asarray(a, dtype=np.float32) for n, a in inputs.items()}
    nc = _get_nc(1)
    shared = ["Wq", "bq", "Wk", "bk", "Wv", "bv", "Wg", "bg"]
    in_maps = []
    for c in range(B):
        m = {"q": ins["q"][c], "k": ins["k"][c], "v": ins["v"][c],
             "mask": ins["mask"][c]}
        for n in shared:
            m[n] = ins[n]
        in_maps.append(m)
    res = run_bass_kernel_spmd(nc, in_maps, list(range(B))).results
    return np.stack([res[c]["out"] for c in range(B)]).astype(np.float32)


# revision 47
# speedup vs baseline: 1.1077x; 1.1077x over previous
"""Trainium2 Bass kernel for nn_CrossAttention (B=8, L=2048, D=1024).

Sharding: data-parallel over batch — each of the 8 NeuronCores handles one
batch element end-to-end (no collectives).

fp8(e4m3) version: all five big matmuls (q/k/v projections, scores, PV,
gate) run in fp8 with MatmulPerfMode.DoubleRow (256-deep contraction per
instruction, ~1.5x bf16 matmul throughput), fp32 PSUM accumulation.
Numerics (validated vs the reference in numpy, rel err ~1.7e-3):
  - weights are pre-scaled by 64 before the fp8 cast (keeps N(0,1/1024)
    entries in the e4m3 normal range); the 1/64 descale is folded into the
    PSUM-evacuation op's scale.
  - P = exp(S/sqrt(D) - 1.5): the offset keeps exp below the TRN e4m3 max
    of 240 and cancels exactly in the softmax normalization.
  - bv is folded into the vp projection: (P@(vp+bv))/l = x + bv exactly.
All intermediates (kp^T, vp, Wg, P, x) stay SBUF-resident — no DRAM
bounce. Activations are kept feature-on-partition so every matmul
contracts along partitions; inputs are transposed on the PE after a bf16
convert (fp8 written on the PSUM evacuation), and the result is
transposed back and fused with mask + residual (f32) on the way out.
Matmul loops are "paired": each DoubleRow stationary is reused by two
back-to-back moving streams (two query chunks / two row groups), halving
the LDWEIGHTS traffic, which is the main fp8 DoubleRow overhead.
"""

import numpy as np

import concourse.bass as bass
import concourse.bacc as bacc
import concourse.tile as tile
import concourse.mybir as mybir
from concourse.bass_utils import run_bass_kernel_spmd
from concourse.masks import make_identity

f32 = mybir.dt.float32
bf16 = mybir.dt.bfloat16
fp8 = mybir.dt.float8e4
AF = mybir.ActivationFunctionType
ALU = mybir.AluOpType
DR = mybir.MatmulPerfMode.DoubleRow

B = 8
L = 2048
D = 1024
P = 128
NT = D // P        # 8 feature tiles
JT = L // P        # 16 key tiles
IC = 512           # query chunk (free dim of moving operands)
NCHUNK = L // IC   # 4
GROUP = IC // P    # 4 row-tiles per chunk/group
NPAIR = NT // 2    # 4 feature-tile pairs (DoubleRow contracts 256)
JPAIR = JT // 2    # 8 key-tile pairs
SCALE = 1.0 / np.sqrt(np.float32(D))
WS = 64.0          # weight pre-scale before fp8 cast
EXP_OFF = -1.5     # exp bias: keeps P under the TRN e4m3 max (240)


def build_kernel(n_iters: int = 1, hw_loop: bool = False):
    nc = bacc.Bacc("TRN2", target_bir_lowering=False, debug=False)

    q_d = nc.dram_tensor("q", [L, D], f32, kind="ExternalInput").ap()
    k_d = nc.dram_tensor("k", [L, D], f32, kind="ExternalInput").ap()
    v_d = nc.dram_tensor("v", [L, D], f32, kind="ExternalInput").ap()
    mask_d = nc.dram_tensor("mask", [L], f32, kind="ExternalInput").ap()
    Wq_d = nc.dram_tensor("Wq", [D, D], f32, kind="ExternalInput").ap()
    bq_d = nc.dram_tensor("bq", [D], f32, kind="ExternalInput").ap()
    Wk_d = nc.dram_tensor("Wk", [D, D], f32, kind="ExternalInput").ap()
    bk_d = nc.dram_tensor("bk", [D], f32, kind="ExternalInput").ap()
    Wv_d = nc.dram_tensor("Wv", [D, D], f32, kind="ExternalInput").ap()
    bv_d = nc.dram_tensor("bv", [D], f32, kind="ExternalInput").ap()
    Wg_d = nc.dram_tensor("Wg", [2 * D, D], f32, kind="ExternalInput").ap()
    bg_d = nc.dram_tensor("bg", [D], f32, kind="ExternalInput").ap()
    out_d = nc.dram_tensor("out", [L, D], f32, kind="ExternalOutput").ap()

    from contextlib import ExitStack, nullcontext
    with tile.TileContext(nc) as tc:
        with ExitStack() as stack:
            pool = lambda *a, **kw: stack.enter_context(tc.tile_pool(*a, **kw))
            cst = pool(name="cst", bufs=1)
            fw32 = pool(name="fw32", bufs=2)      # [128,1024]f32 x2 =  8KB
            natp = pool(name="nat", bufs=2)       # [128,1024]f32 x2 =  8KB
            nat8 = pool(name="nat8", bufs=5)      # [128,1024]bf16x5 = 10KB
            ktp = pool(name="kt", bufs=3)         # [128,8,512]fp8 x3 = 12KB
            wq8p = pool(name="wq8", bufs=1)       # [128,8,1024]fp8  =  8KB
            wk8p = pool(name="wk8", bufs=1)       # 8KB
            wv8p = pool(name="wv8", bufs=1)       # 8KB
            wg8p = pool(name="wg8", bufs=1)       # [128,16,1024]fp8 = 16KB
            kpTp = pool(name="kpT", bufs=1)       # [128,8,2048]fp8  = 16KB
            vpp = pool(name="vp", bufs=1)         # [128,16,1024]fp8 = 16KB
            qpTp = pool(name="qpT", bufs=1)       # [128,8,2048]fp8  = 16KB
            ptp = pool(name="pt", bufs=2)         # [128,16,512]fp8 x2 = 16KB
            xtp = pool(name="xt", bufs=2)         # [128,8,512]fp8 x2 =  8KB
            sgp = pool(name="sg", bufs=4)         # [128,512]bf16 x4 =  4KB
            rtp = pool(name="rt", bufs=17)        # [128,512]bf16x17 = 17KB
            q0p = pool(name="q0", bufs=2)         # [128,1024]f32 x2 =  8KB
            oscp = pool(name="osb", bufs=3)       # [128,512]f32 x3  =  6KB
            mscp = pool(name="msc", bufs=2)       # misc f32         =  ~6KB
            psmm = pool(name="ps", bufs=3, space="PSUM")   # 3 banks
            ps8 = pool(name="ps8", bufs=2, space="PSUM")   # input transposes
            psb = pool(name="psb", bufs=2, space="PSUM")   # output transposes
            pslb = pool(name="psl", bufs=1, space="PSUM")  # colsum

            # ---- constants ----
            idb = cst.tile([P, P], bf16, tag="idb")
            make_identity(nc, idb[:])
            # ones for DoubleRow colsum: [128, 2, 16] so the pair-dim byte
            # step (16) satisfies the DoubleRow weight-AP alignment.
            ones_p8 = cst.tile([P, 2, 16], fp8, tag="ones_p8")
            nc.vector.memset(ones_p8[:], 1.0)
            mask_h = cst.tile([P, JT], f32, tag="mask_h")
            nc.sync.dma_start(mask_h[:], mask_d.rearrange("(t p) -> p t", p=P))
            # R = x*(1+tanh) = 2*x*sigmoid(gate), so fold the 0.5 into mask
            nc.vector.tensor_scalar_mul(mask_h[:], mask_h[:], 0.5)
            bq_t = cst.tile([P, NT], f32, tag="bq_t")
            nc.sync.dma_start(bq_t[:], bq_d.rearrange("(t p) -> p t", p=P))
            bk_t = cst.tile([P, NT], f32, tag="bk_t")
            nc.sync.dma_start(bk_t[:], bk_d.rearrange("(t p) -> p t", p=P))
            bg_h = cst.tile([P, NT], f32, tag="bg_h")
            nc.sync.dma_start(bg_h[:], bg_d.rearrange("(t p) -> p t", p=P))
            nc.vector.tensor_scalar_mul(bg_h[:], bg_h[:], 0.5)
            eoff = cst.tile([P, 1], f32, tag="eoff")
            nc.vector.memset(eoff[:], EXP_OFF)
            one_pp = cst.tile([P, 1], f32, tag="one_pp")
            nc.vector.memset(one_pp[:], 1.0)
            # bv broadcast along free dim (vp_sb is row-on-partition, so the
            # per-feature bias varies along free): bv_bc[p, d] = bv[d]
            bv_r = cst.tile([1, D], f32, tag="bv_r")
            nc.sync.dma_start(bv_r[:], bv_d.rearrange("(o d) -> o d", o=1))
            bv_bc = cst.tile([P, D], f32, tag="bv_bc")
            nc.gpsimd.partition_broadcast(bv_bc[:], bv_r[:])

            Wq8 = wq8p.tile([P, NT, D], fp8, tag="w")
            Wk8 = wk8p.tile([P, NT, D], fp8, tag="w")
            Wv8 = wv8p.tile([P, NT, D], fp8, tag="w")
            Wg8 = wg8p.tile([P, 2 * NT, D], fp8, tag="w")
            kpT_sb = kpTp.tile([P, NT, L], fp8, tag="kpT")
            vp_sb = vpp.tile([P, JT, D], fp8, tag="vp")
            qpT_sb = qpTp.tile([P, NT, L], fp8, tag="qpT")

            def load_w8(dst, w_d, ntiles):
                # DMA f32 weight rows, cast to fp8 with the x64 pre-scale
                for r in range(ntiles):
                    w32 = fw32.tile([P, D], f32, tag="fw32")
                    nc.scalar.dma_start(w32[:], w_d[r * P:(r + 1) * P, :])
                    nc.vector.tensor_scalar_mul(dst[:, r, :], w32[:], WS)

            def load_group_T(src_d, g, dst_kt):
                """Rows [g*512,(g+1)*512) of src_d -> dst_kt[p, et, j] =
                fp8(src[g*512+j, et*128+p]) (feature on partition), via bf16
                PE transposes; fp8 conversion happens on the PSUM->SBUF
                evacuation copy."""
                nats = []
                for t in range(GROUP):
                    n32 = natp.tile([P, D], f32, tag="nat")
                    r0 = (g * GROUP + t) * P
                    nc.sync.dma_start(n32[:], src_d[r0:r0 + P, :])
                    n8 = nat8.tile([P, D], bf16, tag="nat8")
                    if t == 3:
                        nc.scalar.copy(n8[:], n32[:])
                    else:
                        nc.gpsimd.tensor_copy(n8[:], n32[:])
                    nats.append(n8)
                for et in range(NT):
                    pt_ps = ps8.tile([P, IC], bf16, tag="t8")
                    for t in range(GROUP):
                        nc.tensor.transpose(
                            pt_ps[:, t * P:(t + 1) * P],
                            nats[t][:, et * P:(et + 1) * P], idb[:])
                    nc.scalar.copy(dst_kt[:, et, :], pt_ps[:])

            def body_ctx():
                if hw_loop and n_iters > 1:
                    return tc.For_i(0, n_iters, 1)
                return nullcontext()

            for _ in range(1 if hw_loop else n_iters):
              with body_ctx():
                # ===== k projection -> kpT_sb (paired groups) =====
                load_w8(Wk8, Wk_d, NT)
                for gp in range(NCHUNK // 2):
                    kts = []
                    for g2 in range(2):
                        kt = ktp.tile([P, NT, IC], fp8, tag="kt",
                                      name=f"kt{g2}")
                        load_group_T(k_d, 2 * gp + g2, kt)
                        kts.append(kt)
                    for nt in range(NT):
                        pss = [psmm.tile([P, IC], f32, tag="mm",
                                         name=f"mm{g2}") for g2 in range(2)]
                        for m in range(NPAIR):
                            for g2 in range(2):
                                nc.tensor.matmul(
                                    pss[g2][:],
                                    Wk8[:, 2 * m:2 * m + 2, nt * P:(nt + 1) * P],
                                    kts[g2][:, 2 * m:2 * m + 2, :],
                                    start=(m == 0), stop=(m == NPAIR - 1),
                                    perf_mode=DR)
                        for g2 in range(2):
                            g = 2 * gp + g2
                            nc.vector.tensor_scalar(
                                kpT_sb[:, nt, g * IC:(g + 1) * IC],
                                pss[g2][:], 1.0 / WS, bk_t[:, nt:nt + 1],
                                ALU.mult, ALU.add)

                # ===== v projection -> vp_sb (row on partition) =====
                load_w8(Wv8, Wv_d, NT)
                for g in range(NCHUNK):
                    vt = ktp.tile([P, NT, IC], fp8, tag="kt")
                    load_group_T(v_d, g, vt)
                    for rt_i in range(GROUP):
                        pss = [psmm.tile([P, IC], f32, tag="mm",
                                         name=f"mmv{fh}") for fh in range(2)]
                        for m in range(NPAIR):
                            for fh in range(2):
                                nc.tensor.matmul(
                                    pss[fh][:],
                                    vt[:, 2 * m:2 * m + 2, rt_i * P:(rt_i + 1) * P],
                                    Wv8[:, 2 * m:2 * m + 2, fh * IC:(fh + 1) * IC],
                                    start=(m == 0), stop=(m == NPAIR - 1),
                                    perf_mode=DR)
                        for fh in range(2):
                            # bv folded in: (P@(vp+bv))/l = x + bv exactly
                            nc.vector.scalar_tensor_tensor(
                                vp_sb[:, g * GROUP + rt_i, fh * IC:(fh + 1) * IC],
                                pss[fh][:], 1.0 / WS,
                                bv_bc[:, fh * IC:(fh + 1) * IC],
                                ALU.mult, ALU.add)

                load_w8(Wg8, Wg_d, 2 * NT)

                # ===== q projection -> qpT_sb (paired groups) =====
                load_w8(Wq8, Wq_d, NT)
                for gp in range(NCHUNK // 2):
                    qts = []
                    for g2 in range(2):
                        qt = ktp.tile([P, NT, IC], fp8, tag="kt",
                                      name=f"qt{g2}")
                        load_group_T(q_d, 2 * gp + g2, qt)
                        qts.append(qt)
                    for nt in range(NT):
                        pss = [psmm.tile([P, IC], f32, tag="mm",
                                         name=f"mmq{g2}") for g2 in range(2)]
                        for m in range(NPAIR):
                            for g2 in range(2):
                                nc.tensor.matmul(
                                    pss[g2][:],
                                    Wq8[:, 2 * m:2 * m + 2, nt * P:(nt + 1) * P],
                                    qts[g2][:, 2 * m:2 * m + 2, :],
                                    start=(m == 0), stop=(m == NPAIR - 1),
                                    perf_mode=DR)
                        for g2 in range(2):
                            g = 2 * gp + g2
                            nc.vector.tensor_scalar(
                                qpT_sb[:, nt, g * IC:(g + 1) * IC],
                                pss[g2][:], 1.0 / WS, bq_t[:, nt:nt + 1],
                                ALU.mult, ALU.add)

                # ===== fused attention, two query chunks per pass =====
                for icp in range(NCHUNK // 2):
                    qsls = [slice((2 * icp + qc) * IC, (2 * icp + qc + 1) * IC)
                            for qc in range(2)]
                    # --- scores S^T + exp -> pts (fp8) ---
                    pts = [ptp.tile([P, JT, IC], fp8, tag="pt",
                                    name=f"pt{qc}") for qc in range(2)]
                    for jt in range(JT):
                        pss = [psmm.tile([P, IC], f32, tag="mm",
                                         name=f"mms{qc}") for qc in range(2)]
                        for m in range(NPAIR):
                            for qc in range(2):
                                nc.tensor.matmul(
                                    pss[qc][:],
                                    kpT_sb[:, 2 * m:2 * m + 2, jt * P:(jt + 1) * P],
                                    qpT_sb[:, 2 * m:2 * m + 2, qsls[qc]],
                                    start=(m == 0), stop=(m == NPAIR - 1),
                                    perf_mode=DR)
                        for qc in range(2):
                            nc.scalar.activation(pts[qc][:, jt, :],
                                                 pss[qc][:], AF.Exp,
                                                 bias=eoff[:],
                                                 scale=float(SCALE))

                    # --- l = colsum(P), r = 1/l, broadcast (per chunk) ---
                    rbcs = []
                    for qc in range(2):
                        ps_l = pslb.tile([1, IC], f32, tag="lb")
                        for jj in range(JPAIR):
                            nc.tensor.matmul(ps_l[:], ones_p8[:, :, 0:1],
                                             pts[qc][:, 2 * jj:2 * jj + 2, :],
                                             start=(jj == 0),
                                             stop=(jj == JPAIR - 1),
                                             perf_mode=DR)
                        r_sb = mscp.tile([1, IC], f32, tag="r_sb", bufs=2)
                        nc.vector.reciprocal(r_sb[:], ps_l[:])
                        rbc = mscp.tile([P, IC], f32, tag="rbc", bufs=2)
                        nc.gpsimd.partition_broadcast(rbc[:], r_sb[:])
                        rbcs.append(rbc)

                    # --- x = (P @ (vp+bv)) * r ---
                    xt8s = [xtp.tile([P, NT, IC], fp8, tag="xt",
                                     name=f"xt{qc}") for qc in range(2)]
                    for dt in range(NT):
                        pss = [psmm.tile([P, IC], f32, tag="mm",
                                         name=f"mmx{qc}") for qc in range(2)]
                        for jj in range(JPAIR):
                            for qc in range(2):
                                nc.tensor.matmul(
                                    pss[qc][:],
                                    vp_sb[:, 2 * jj:2 * jj + 2, dt * P:(dt + 1) * P],
                                    pts[qc][:, 2 * jj:2 * jj + 2, :],
                                    start=(jj == 0), stop=(jj == JPAIR - 1),
                                    perf_mode=DR)
                        for qc in range(2):
                            nc.vector.tensor_mul(xt8s[qc][:, dt, :],
                                                 pss[qc][:], rbcs[qc][:])

                    # --- gate + tanh; R inline so sg dies quickly ---
                    rtss = [[], []]
                    for nt in range(NT):
                        pss = [psmm.tile([P, IC], f32, tag="mm",
                                         name=f"mmg{qc}") for qc in range(2)]
                        for m in range(NPAIR):
                            for qc in range(2):
                                nc.tensor.matmul(
                                    pss[qc][:],
                                    Wg8[:, 2 * m:2 * m + 2, nt * P:(nt + 1) * P],
                                    qpT_sb[:, 2 * m:2 * m + 2, qsls[qc]],
                                    start=(m == 0), stop=False, perf_mode=DR)
                        for m in range(NPAIR):
                            for qc in range(2):
                                nc.tensor.matmul(
                                    pss[qc][:],
                                    Wg8[:, NT + 2 * m:NT + 2 * m + 2, nt * P:(nt + 1) * P],
                                    xt8s[qc][:, 2 * m:2 * m + 2, :],
                                    start=False, stop=(m == NPAIR - 1),
                                    perf_mode=DR)
                        for qc in range(2):
                            sg = sgp.tile([P, IC], bf16, tag="sg")
                            nc.scalar.activation(sg[:], pss[qc][:], AF.Tanh,
                                                 bias=bg_h[:, nt:nt + 1],
                                                 scale=0.5 / WS)
                            r_t = rtp.tile([P, IC], bf16, tag="rt")
                            nc.vector.scalar_tensor_tensor(
                                r_t[:], sg[:], one_pp[:, 0:1],
                                xt8s[qc][:, nt, :], ALU.add, ALU.mult)
                            rtss[qc].append(r_t)

                    # --- transpose back, apply mask, add residual, store ---
                    for qc in range(2):
                        ic = 2 * icp + qc
                        rts = rtss[qc]
                        for t in range(GROUP):
                            it = ic * GROUP + t
                            q0 = q0p.tile([P, D], f32, tag="q0")
                            nc.scalar.dma_start(q0[:],
                                                q_d[it * P:(it + 1) * P, :])
                            for mh in range(2):
                                ps_n = psb.tile([P, IC], bf16, tag="tb")
                                for m4 in range(4):
                                    mm_ = mh * 4 + m4
                                    nc.tensor.transpose(
                                        ps_n[:, m4 * P:(m4 + 1) * P],
                                        rts[mm_][:, t * P:(t + 1) * P],
                                        idb[:])
                                osb = oscp.tile([P, IC], f32, tag="osb")
                                nc.vector.scalar_tensor_tensor(
                                    osb[:], ps_n[:], mask_h[:, it:it + 1],
                                    q0[:, mh * IC:(mh + 1) * IC],
                                    ALU.mult, ALU.add)
                                nc.gpsimd.dma_start(
                                    out_d[it * P:(it + 1) * P,
                                          mh * IC:(mh + 1) * IC], osb[:])

    nc.compile()
    return nc


_CACHE = {}


def _get_nc(n_iters=1):
    if n_iters not in _CACHE:
        _CACHE[n_iters] = build_kernel(n_iters)
    return _CACHE[n_iters]


def kernel(**inputs):
    ins = {n: np.asarray(a, dtype=np.float32) for n, a in inputs.items()}
    nc = _get_nc(1)
    shared = ["Wq", "bq", "Wk", "bk", "Wv", "bv", "Wg", "bg"]
    in_maps = []
    for c in range(B):
        m = {"q": ins["q"][c], "k": ins["k"][c], "v": ins["v"][c],
             "mask": ins["mask"][c]}
        for n in shared:
            m[n] = ins[n]
        in_maps.append(m)
    res = run_bass_kernel_spmd(nc, in_maps, list(range(B))).results
    return np.stack([res[c]["out"] for c in range(B)]).astype(np.float32)
